# revision 4
# baseline (speedup 1.0000x reference)
"""Trainium2 Bass kernel for NeuralKNN (soft k-nearest-neighbors).

Reference computation (per batch element b):
    sims  = -(q . K) / sqrt(D)                      [N]
    a0    = softmax(sims)                           [N]
    repeat 16x:  w_k = softmax(a / 0.1); a += log1p(-w_k)
    out[k, f] = sum_n w_k[n] * V[f, n]              [16, F]

Strategy: pure data-parallel over B=8 -> one batch element per NeuronCore,
no collectives. Per core:
  phase 1: stream K (bf16) through the PE as stationary weights against the
           query vector -> sims laid out [128, 782] (n = t*128 + p).
  phase 2: 16 softmax iterations on [128, 782] in SBUF. Row sums come free
           via activation accum_out; cross-partition sum + broadcast via a
           ones[128,128] matmul. Stores E'_k = exp(10*a)-1 in bf16 (keeps
           precision since E ~= 1) plus a ones column.
  phase 3: V arrives block-transposed via DMA xbar transpose (bf16) as
           [n,f] tiles; one accumulating matmul per 128-n tile:
           psum[f, 0:17] += Vt.T @ [E'_0..E'_15 | 1].
  final:   out[f,k] = (psum[:,k] + psum[:,16]) * (1/S_k); host transposes.

Inputs are cast to bf16 on the host (error through the double-softmax is
~1e-5 relative; value quantization ~0.2% relative, well within tolerance)
and zero-padded from N=100000 to 100096 = 782*128.
"""

import sys

sys.path.insert(0, "/opt/trn_rl_repo")

import numpy as np
import ml_dtypes

B, D, N, F = 8, 128, 100000, 128
KK = 16
TEMP = 0.1
NT = (N + 127) // 128          # 782 n-tiles
NP = NT * 128                  # 100096 padded N
SIMS_SCALE = float(-1.0 / np.sqrt(D))
N_CORES = 8

KCH = 2048                     # keys DMA chunk (columns)
VCH = 48                       # value-transpose DMA chunk (128-col tiles)
VRING_BUFS = 10
PAD_P0 = N - (NT - 1) * 128    # first padded partition in the last tile (32)

_BF16 = ml_dtypes.bfloat16
_BUILD_CACHE = {}


def _build_nc():
    import concourse.bass as bass  # noqa: F401
    import concourse.mybir as mybir
    import concourse.tile as tile
    from concourse import bacc

    f32 = mybir.dt.float32
    bf16 = mybir.dt.bfloat16
    AF = mybir.ActivationFunctionType
    ALU = mybir.AluOpType

    nc = bacc.Bacc("TRN2", target_bir_lowering=False, debug=False)

    q_d = nc.dram_tensor("query", [D, 1], bf16, kind="ExternalInput")
    k_d = nc.dram_tensor("keys", [D, NP], bf16, kind="ExternalInput")
    v_d = nc.dram_tensor("values", [F, NP], bf16, kind="ExternalInput")
    o_d = nc.dram_tensor("out", [F, KK], f32, kind="ExternalOutput")

    with tile.TileContext(nc) as tc:
        with (
            tc.tile_pool(name="const", bufs=1) as constp,
            tc.tile_pool(name="work", bufs=1) as workp,
            tc.tile_pool(name="kring", bufs=4) as kring,
            tc.tile_pool(name="vring", bufs=VRING_BUFS) as vring,
            tc.tile_pool(name="ps_sims", bufs=2, space="PSUM") as ps_sims_p,
            tc.tile_pool(name="ps_small", bufs=2, space="PSUM") as ps_small_p,
            tc.tile_pool(name="ps_out", bufs=1, space="PSUM") as ps_out_p,
        ):
            q_sb = constp.tile([128, 1], bf16)
            nc.sync.dma_start(q_sb[:, :], q_d[:, :])
            ones = constp.tile([128, 128], f32)
            nc.vector.memset(ones[:, :], 1.0)

            sims = workp.tile([128, NT], f32)
            e0_scr = workp.tile([128, NT], f32)
            E_a = workp.tile([128, NT], f32)
            E_b = workp.tile([128, NT], f32)
            t_scr = workp.tile([128, NT], f32)
            m_scr = workp.tile([128, NT], f32)
            Wp = workp.tile([128, NT, KK + 1], bf16)
            rs = workp.tile([128, 1], f32)
            recip = workp.tile([128, 1], f32)
            sfix = workp.tile([128, 1], f32)
            a_sc = workp.tile([128, 1], f32)
            b_sc = workp.tile([128, 1], f32)
            rvec = workp.tile([128, KK], f32)
            rb_sb = workp.tile([128, KK], f32)
            out17 = workp.tile([128, KK + 1], f32)
            out_sb = workp.tile([128, KK], f32)

            # ----- Value DMA-transposes (scalar engine ucode; V has no deps,
            # so these prefetch from t=0 while keys stream on sync). The
            # first batch is emitted before the phase-2 activations so the
            # in-order scalar sequencer doesn't park them behind compute.
            vchunks = [(s, min(VCH, NT - s)) for s in range(0, NT, VCH)]
            vt_tiles = {}

            def emit_vt(ci):
                s, nt_chunk = vchunks[ci]
                vt = vring.tile([128, VCH, 128], bf16, tag="vt")
                vt_tiles[ci] = vt
                nc.scalar.dma_start_transpose(
                    vt[:, 0:nt_chunk, :],
                    v_d[:, s * 128 : (s + nt_chunk) * 128],
                )

            N_VT_EARLY = min(VRING_BUFS, len(vchunks))
            for ci in range(N_VT_EARLY):
                emit_vt(ci)

            # ---------------- Phase 1: sims ----------------
            ps = None
            for s in range(0, NP, KCH):
                w = min(KCH, NP - s)
                kt = kring.tile([128, KCH], bf16, tag="kt")
                nc.sync.dma_start(kt[:, 0:w], k_d[:, s : s + w])
                for j in range(w // 128):
                    t = s // 128 + j
                    c = t % 512
                    if c == 0:
                        ps = ps_sims_p.tile([128, 512], f32, tag="pss")
                    nc.tensor.matmul(
                        ps[:, c : c + 1],
                        kt[:, j * 128 : (j + 1) * 128],
                        q_sb[:, 0:1],
                        start=True,
                        stop=True,
                    )
                    if c == 511 or t == NT - 1:
                        base = (t // 512) * 512
                        nc.vector.tensor_copy(
                            sims[:, base : t + 1], ps[:, 0 : c + 1]
                        )
            # mark padded positions so exp() kills them (<=32 partitions per
            # memset when base partition is nonzero)
            for p0 in range(PAD_P0, 128, 32):
                nc.vector.memset(sims[p0 : p0 + 32, NT - 1 : NT], 1.0e5)

            # ---------------- Phase 2: iterated softmax ----------------
            # All heavy per-iteration work is on the DVE via the polynomial
            # identity  E_{k+1} = E_k*(1-w)^10 ~= E_k*(1 - 10w + 45w^2),
            # w = E_k/S_k <= ~1e-5 (truncation error ~1e-13, far below f32).
            # ACT only computes e0 and E_1; GpSimd stores W'_k = E_k - 1.
            # e0 = exp(-sims/sqrt(D)); rs = rowsum(e0)
            nc.scalar.activation(
                e0_scr[:, :], sims[:, :], AF.Exp,
                bias=0.0, scale=SIMS_SCALE, accum_out=rs[:, 0:1],
            )
            psS = ps_small_p.tile([128, 1], f32, tag="psS")
            nc.tensor.matmul(psS[:, 0:1], ones[:, :], rs[:, 0:1], start=True, stop=True)
            nc.vector.reciprocal(recip[:, 0:1], psS[:, 0:1])
            # E_1 = exp(10 * e0 / S0): scale AP = 10*r0
            nc.vector.tensor_scalar_mul(sfix[:, 0:1], recip[:, 0:1], 10.0)
            nc.scalar.activation(
                E_a[:, :], e0_scr[:, :], AF.Exp,
                bias=0.0, scale=sfix[:, 0:1], accum_out=rs[:, 0:1],
            )
            # padded positions: E=0 forever (w=0 fixpoint). exp(0)=1 was
            # summed into rs for 96 pad elements -> S_1 fix below.
            for p0 in range(PAD_P0, 128, 32):
                nc.vector.memset(E_a[p0 : p0 + 32, NT - 1 : NT], 0.0)
            # the "sum of V" column
            nc.vector.memset(Wp[:, :, KK], 1.0)

            cur, nxt = E_a, E_b
            n_pad = 128 - PAD_P0
            for k in range(KK):
                psS = ps_small_p.tile([128, 1], f32, tag="psS")
                nc.tensor.matmul(
                    psS[:, 0:1], ones[:, :], rs[:, 0:1], start=True, stop=True
                )
                if k == 0:
                    nc.vector.tensor_scalar_add(sfix[:, 0:1], psS[:, 0:1], -float(n_pad))
                    nc.vector.reciprocal(recip[:, 0:1], sfix[:, 0:1])
                else:
                    nc.vector.reciprocal(recip[:, 0:1], psS[:, 0:1])
                nc.vector.tensor_copy(rvec[0:1, k : k + 1], recip[0:1, 0:1])
                # store W'_k = E_k - 1 in bf16 (GpSimd: off the DVE chain)
                nc.gpsimd.tensor_scalar_add(Wp[:, :, k], cur[:, :], -1.0)
                if k < KK - 1:
                    # u = 1 + a*E + b*E^2,  a = -10/S, b = 45/S^2
                    nc.vector.tensor_scalar_mul(a_sc[:, 0:1], recip[:, 0:1], -10.0)
                    nc.vector.scalar_tensor_tensor(
                        b_sc[:, 0:1], recip[:, 0:1], 45.0, recip[:, 0:1],
                        op0=ALU.mult, op1=ALU.mult,
                    )
                    nc.vector.tensor_scalar(
                        t_scr[:, :], cur[:, :], b_sc[:, 0:1], a_sc[:, 0:1],
                        op0=ALU.mult, op1=ALU.add,
                    )
                    nc.vector.tensor_mul(m_scr[:, :], t_scr[:, :], cur[:, :])
                    # E_{k+1} = (m+1)*E ; rowsums -> rs
                    nc.vector.scalar_tensor_tensor(
                        nxt[:, :], m_scr[:, :], 1.0, cur[:, :],
                        op0=ALU.add, op1=ALU.mult, accum_out=rs[:, 0:1],
                    )
                    cur, nxt = nxt, cur

            # broadcast 1/S_k across partitions: [128, KK]
            psB = ps_small_p.tile([128, KK], f32, tag="psB")
            nc.tensor.matmul(
                psB[:, :], ones[0:1, :], rvec[0:1, :], start=True, stop=True
            )
            nc.vector.tensor_copy(rb_sb[:, :], psB[:, :])

            # ---------------- Phase 3: weighted sum of values ----------------
            for ci in range(N_VT_EARLY, len(vchunks)):
                emit_vt(ci)
            ps_out = ps_out_p.tile([128, KK + 1], f32)
            for ci, (s, nt_chunk) in enumerate(vchunks):
                vt = vt_tiles[ci]
                for j in range(nt_chunk):
                    t = s + j
                    nc.tensor.matmul(
                        ps_out[:, :],
                        vt[:, j, :],
                        Wp[:, t, :],
                        start=(t == 0),
                        stop=(t == NT - 1),
                    )

            # ---------------- Final combine ----------------
            nc.vector.tensor_copy(out17[:, :], ps_out[:, :])
            nc.vector.scalar_tensor_tensor(
                out_sb[:, :],
                out17[:, 0:KK],
                out17[:, KK : KK + 1],
                rb_sb[:, :],
                op0=ALU.add,
                op1=ALU.mult,
            )
            nc.sync.dma_start(o_d[:, :], out_sb[:, :])

    nc.compile()
    return nc


def get_nc():
    if "nc" not in _BUILD_CACHE:
        _BUILD_CACHE["nc"] = _build_nc()
    return _BUILD_CACHE["nc"]


def make_in_maps(query, keys, values):
    in_maps = []
    for b in range(query.shape[0]):
        q = np.ascontiguousarray(query[b].astype(_BF16).reshape(D, 1))
        k = np.zeros((D, NP), _BF16)
        k[:, :N] = keys[b].astype(_BF16)
        v = np.zeros((F, NP), _BF16)
        v[:, :N] = values[b].astype(_BF16)
        in_maps.append({"query": q, "keys": k, "values": v})
    return in_maps


def run(query, keys, values, trace=False):
    nc = get_nc()
    from concourse.bass_utils import run_bass_kernel_spmd

    in_maps = make_in_maps(query, keys, values)
    res = run_bass_kernel_spmd(
        nc, in_maps, core_ids=list(range(N_CORES)), trace=trace
    )
    out = np.stack(
        [np.asarray(r["out"], dtype=np.float32).T for r in res.results], axis=0
    )
    return out, res


def kernel(query, keys, values):
    out, _ = run(query, keys, values, trace=False)
    return out


# revision 9
# speedup vs baseline: 1.3725x; 1.3725x over previous
"""Trainium2 Bass kernel for NeuralKNN (soft k-nearest-neighbors).

Reference computation (per batch element b):
    sims  = -(q . K) / sqrt(D)                      [N]
    a0    = softmax(sims)                           [N]
    repeat 16x:  w_k = softmax(a / 0.1); a += log1p(-w_k)
    out[k, f] = sum_n w_k[n] * V[f, n]              [16, F]

Strategy: pure data-parallel over B=8 -> one batch element per NeuronCore,
no collectives. Per core:
  phase 1: stream K (bf16) through the PE as stationary weights against the
           query vector -> sims laid out [128, 782] (n = t*128 + p).
  phase 2: 16 softmax iterations on [128, 782] in SBUF. Row sums come free
           via activation accum_out; cross-partition sum + broadcast via a
           ones[128,128] matmul. Stores E'_k = exp(10*a)-1 in bf16 (keeps
           precision since E ~= 1) plus a ones column.
  phase 3: V arrives block-transposed via DMA xbar transpose (bf16) as
           [n,f] tiles; one accumulating matmul per 128-n tile:
           psum[f, 0:17] += Vt.T @ [E'_0..E'_15 | 1].
  final:   out[f,k] = (psum[:,k] + psum[:,16]) * (1/S_k); host transposes.

Inputs are cast to bf16 on the host (error through the double-softmax is
~1e-5 relative; value quantization ~0.2% relative, well within tolerance)
and zero-padded from N=100000 to 100096 = 782*128.
"""

import sys

sys.path.insert(0, "/opt/trn_rl_repo")

import numpy as np
import ml_dtypes

B, D, N, F = 8, 128, 100000, 128
KK = 16
TEMP = 0.1
NT = (N + 127) // 128          # 782 n-tiles
NP = NT * 128                  # 100096 padded N
SIMS_SCALE = float(-1.0 / np.sqrt(D))
N_CORES = 8

KCH = 2048                     # keys DMA chunk (columns)
VCH = 48                       # value-transpose DMA chunk (128-col tiles)
VRING_BUFS = 11
PAD_P0 = N - (NT - 1) * 128    # first padded partition in the last tile (32)

_BF16 = ml_dtypes.bfloat16
_BUILD_CACHE = {}


def _build_nc():
    import concourse.bass as bass  # noqa: F401
    import concourse.mybir as mybir
    import concourse.tile as tile
    from concourse import bacc

    f32 = mybir.dt.float32
    bf16 = mybir.dt.bfloat16
    AF = mybir.ActivationFunctionType
    ALU = mybir.AluOpType

    nc = bacc.Bacc("TRN2", target_bir_lowering=False, debug=False)

    q_d = nc.dram_tensor("query", [D, 1], bf16, kind="ExternalInput")
    k_d = nc.dram_tensor("keys", [D, NP], bf16, kind="ExternalInput")
    v_d = nc.dram_tensor("values", [F, NP], bf16, kind="ExternalInput")
    o_d = nc.dram_tensor("out", [F, KK], f32, kind="ExternalOutput")

    with tile.TileContext(nc) as tc:
        with (
            tc.tile_pool(name="const", bufs=1) as constp,
            tc.tile_pool(name="work", bufs=1) as workp,
            tc.tile_pool(name="kring", bufs=3) as kring,
            tc.tile_pool(name="vring", bufs=VRING_BUFS) as vring,
            tc.tile_pool(name="ps_sims", bufs=2, space="PSUM") as ps_sims_p,
            tc.tile_pool(name="ps_small", bufs=2, space="PSUM") as ps_small_p,
            tc.tile_pool(name="ps_out", bufs=1, space="PSUM") as ps_out_p,
        ):
            q_sb = constp.tile([128, 1], bf16)
            nc.sync.dma_start(q_sb[:, :], q_d[:, :])
            ones = constp.tile([128, 128], f32)
            nc.vector.memset(ones[:, :], 1.0)

            sims = workp.tile([128, NT], f32, tag="scrA")
            e0_scr = workp.tile([128, NT], f32, tag="scrB")
            E_a = workp.tile([128, NT], f32)
            E_b = workp.tile([128, NT], f32)
            # t/m scratch reuse the sims/e0 slots (dead after phase-2 setup)
            t_scr = workp.tile([128, NT], f32, tag="scrA")
            m_scr = workp.tile([128, NT], f32, tag="scrB")
            Wp = workp.tile([128, NT, KK + 1], bf16)
            rs = workp.tile([128, 1], f32)
            recip = workp.tile([128, 1], f32)
            sfix = workp.tile([128, 1], f32)
            a_sc = workp.tile([128, 1], f32)
            b_sc = workp.tile([128, 1], f32)
            rvec = workp.tile([128, KK], f32)
            rb_sb = workp.tile([128, KK], f32)
            out17 = workp.tile([128, KK + 1], f32)
            out_sb = workp.tile([128, KK], f32)

            # ----- Value DMA-transposes run on the scalar engine's HWDGE
            # queue, emitted after the phase-2 activations: they fill the
            # vring during phase 2 / phase 3 without stealing HBM bandwidth
            # from the keys stream during phase 1.
            vchunks = [(s, min(VCH, NT - s)) for s in range(0, NT, VCH)]
            vt_tiles = {}

            def emit_vt(ci):
                s, nt_chunk = vchunks[ci]
                vt = vring.tile([128, VCH, 128], bf16, tag="vt")
                vt_tiles[ci] = vt
                nc.scalar.dma_start_transpose(
                    vt[:, 0:nt_chunk, :],
                    v_d[:, s * 128 : (s + nt_chunk) * 128],
                )

            # ---------------- Phase 1: sims ----------------
            ps = None
            for s in range(0, NP, KCH):
                w = min(KCH, NP - s)
                kt = kring.tile([128, KCH], bf16, tag="kt")
                nc.sync.dma_start(kt[:, 0:w], k_d[:, s : s + w])
                for j in range(w // 128):
                    t = s // 128 + j
                    c = t % 512
                    if c == 0:
                        ps = ps_sims_p.tile([128, 512], f32, tag="pss")
                    nc.tensor.matmul(
                        ps[:, c : c + 1],
                        kt[:, j * 128 : (j + 1) * 128],
                        q_sb[:, 0:1],
                        start=True,
                        stop=True,
                    )
                    if c == 511 or t == NT - 1:
                        base = (t // 512) * 512
                        nc.vector.tensor_copy(
                            sims[:, base : t + 1], ps[:, 0 : c + 1]
                        )
            # mark padded positions so exp() kills them (<=32 partitions per
            # memset when base partition is nonzero)
            for p0 in range(PAD_P0, 128, 32):
                nc.vector.memset(sims[p0 : p0 + 32, NT - 1 : NT], 1.0e5)

            # ---------------- Phase 2: iterated softmax ----------------
            # All heavy per-iteration work is on the DVE via the polynomial
            # identity  E_{k+1} = E_k*(1-w)^10 ~= E_k*(1 - 10w + 45w^2),
            # w = E_k/S_k <= ~1e-5 (truncation error ~1e-13, far below f32).
            # ACT only computes e0 and E_1; GpSimd stores W'_k = E_k - 1.
            # e0 = exp(-sims/sqrt(D)); rs = rowsum(e0)
            nc.scalar.activation(
                e0_scr[:, :], sims[:, :], AF.Exp,
                bias=0.0, scale=SIMS_SCALE, accum_out=rs[:, 0:1],
            )
            psS = ps_small_p.tile([128, 1], f32, tag="psS")
            nc.tensor.matmul(psS[:, 0:1], ones[:, :], rs[:, 0:1], start=True, stop=True)
            nc.vector.reciprocal(recip[:, 0:1], psS[:, 0:1])
            # E_1 = exp(10 * e0 / S0): scale AP = 10*r0
            nc.vector.tensor_scalar_mul(sfix[:, 0:1], recip[:, 0:1], 10.0)
            nc.scalar.activation(
                E_a[:, :], e0_scr[:, :], AF.Exp,
                bias=0.0, scale=sfix[:, 0:1], accum_out=rs[:, 0:1],
            )
            # padded positions: E=0 forever (w=0 fixpoint). exp(0)=1 was
            # summed into rs for 96 pad elements -> S_1 fix below.
            for p0 in range(PAD_P0, 128, 32):
                nc.vector.memset(E_a[p0 : p0 + 32, NT - 1 : NT], 0.0)
            # the "sum of V" column
            nc.vector.memset(Wp[:, :, KK], 1.0)

            # V transposes start here: scalar engine is done with compute,
            # phase 2 below is DVE-only.
            for ci in range(len(vchunks)):
                emit_vt(ci)

            cur, nxt = E_a, E_b
            n_pad = 128 - PAD_P0
            for k in range(KK):
                psS = ps_small_p.tile([128, 1], f32, tag="psS")
                nc.tensor.matmul(
                    psS[:, 0:1], ones[:, :], rs[:, 0:1], start=True, stop=True
                )
                # W'_k = E_k - 1 (bf16); DVE does this while the PE reduces,
                # hiding the cross-partition round trip.
                nc.vector.tensor_scalar_add(Wp[:, :, k], cur[:, :], -1.0)
                if k == 0:
                    nc.vector.tensor_scalar_add(sfix[:, 0:1], psS[:, 0:1], -float(n_pad))
                    nc.vector.reciprocal(recip[:, 0:1], sfix[:, 0:1])
                else:
                    nc.vector.reciprocal(recip[:, 0:1], psS[:, 0:1])
                nc.vector.tensor_copy(rvec[0:1, k : k + 1], recip[0:1, 0:1])
                if k < KK - 1:
                    # u = 1 + a*E + b*E^2,  a = -10/S, b = 45/S^2 = 0.45*a*a
                    nc.vector.tensor_scalar_mul(a_sc[:, 0:1], recip[:, 0:1], -10.0)
                    nc.vector.tensor_mul(b_sc[:, 0:1], a_sc[:, 0:1], a_sc[:, 0:1])
                    nc.vector.tensor_scalar_mul(b_sc[:, 0:1], b_sc[:, 0:1], 0.45)
                    nc.vector.tensor_scalar(
                        t_scr[:, :], cur[:, :], b_sc[:, 0:1], a_sc[:, 0:1],
                        op0=ALU.mult, op1=ALU.add,
                    )
                    nc.vector.tensor_mul(m_scr[:, :], t_scr[:, :], cur[:, :])
                    # E_{k+1} = (m+1)*E ; rowsums -> rs
                    nc.vector.scalar_tensor_tensor(
                        nxt[:, :], m_scr[:, :], 1.0, cur[:, :],
                        op0=ALU.add, op1=ALU.mult, accum_out=rs[:, 0:1],
                    )
                    cur, nxt = nxt, cur

            # broadcast 1/S_k across partitions: [128, KK]
            psB = ps_small_p.tile([128, KK], f32, tag="psB")
            nc.tensor.matmul(
                psB[:, :], ones[0:1, :], rvec[0:1, :], start=True, stop=True
            )
            nc.vector.tensor_copy(rb_sb[:, :], psB[:, :])

            # ---------------- Phase 3: weighted sum of values ----------------
            ps_out = ps_out_p.tile([128, KK + 1], f32)
            for ci, (s, nt_chunk) in enumerate(vchunks):
                vt = vt_tiles[ci]
                for j in range(nt_chunk):
                    t = s + j
                    nc.tensor.matmul(
                        ps_out[:, :],
                        vt[:, j, :],
                        Wp[:, t, :],
                        start=(t == 0),
                        stop=(t == NT - 1),
                    )

            # ---------------- Final combine ----------------
            nc.vector.tensor_copy(out17[:, :], ps_out[:, :])
            nc.vector.scalar_tensor_tensor(
                out_sb[:, :],
                out17[:, 0:KK],
                out17[:, KK : KK + 1],
                rb_sb[:, :],
                op0=ALU.add,
                op1=ALU.mult,
            )
            nc.sync.dma_start(o_d[:, :], out_sb[:, :])

    nc.compile()
    return nc


def get_nc():
    if "nc" not in _BUILD_CACHE:
        _BUILD_CACHE["nc"] = _build_nc()
    return _BUILD_CACHE["nc"]


def make_in_maps(query, keys, values):
    in_maps = []
    for b in range(query.shape[0]):
        q = np.ascontiguousarray(query[b].astype(_BF16).reshape(D, 1))
        k = np.zeros((D, NP), _BF16)
        k[:, :N] = keys[b].astype(_BF16)
        v = np.zeros((F, NP), _BF16)
        v[:, :N] = values[b].astype(_BF16)
        in_maps.append({"query": q, "keys": k, "values": v})
    return in_maps


def run(query, keys, values, trace=False):
    nc = get_nc()
    from concourse.bass_utils import run_bass_kernel_spmd

    in_maps = make_in_maps(query, keys, values)
    res = run_bass_kernel_spmd(
        nc, in_maps, core_ids=list(range(N_CORES)), trace=trace
    )
    out = np.stack(
        [np.asarray(r["out"], dtype=np.float32).T for r in res.results], axis=0
    )
    return out, res


def kernel(query, keys, values):
    out, _ = run(query, keys, values, trace=False)
    return out


# revision 11
# speedup vs baseline: 1.8223x; 1.3277x over previous
"""Trainium2 Bass kernel for NeuralKNN (soft k-nearest-neighbors).

Reference computation (per batch element b):
    sims  = -(q . K) / sqrt(D)                      [N]
    a0    = softmax(sims)                           [N]
    repeat 16x:  w_k = softmax(a / 0.1); a += log1p(-w_k)
    out[k, f] = sum_n w_k[n] * V[f, n]              [16, F]

Strategy: pure data-parallel over B=8 -> one batch element per NeuronCore,
no collectives. Per core:
  phase 1: stream K (bf16) through the PE as stationary weights against the
           query vector -> sims laid out [128, 782] (n = t*128 + p).
  phase 2: 16 softmax iterations on [128, 782] in SBUF. Row sums come free
           via activation accum_out; cross-partition sum + broadcast via a
           ones[128,128] matmul. Stores E'_k = exp(10*a)-1 in bf16 (keeps
           precision since E ~= 1) plus a ones column.
  phase 3: V arrives block-transposed via DMA xbar transpose (bf16) as
           [n,f] tiles; one accumulating matmul per 128-n tile:
           psum[f, 0:17] += Vt.T @ [E'_0..E'_15 | 1].
  final:   out[f,k] = (psum[:,k] + psum[:,16]) * (1/S_k); host transposes.

Inputs are cast to bf16 on the host (error through the double-softmax is
~1e-5 relative; value quantization ~0.2% relative, well within tolerance)
and zero-padded from N=100000 to 100096 = 782*128.
"""

import sys

sys.path.insert(0, "/opt/trn_rl_repo")

import numpy as np
import ml_dtypes

B, D, N, F = 8, 128, 100000, 128
KK = 16
TEMP = 0.1
NT = (N + 127) // 128          # 782 n-tiles
NP = NT * 128                  # 100096 padded N
SIMS_SCALE = float(-1.0 / np.sqrt(D))
N_CORES = 8

KCH = 2048                     # keys DMA chunk (columns)
VCH = 48                       # value-transpose DMA chunk (128-col tiles)
VRING_BUFS = 11
PAD_P0 = N - (NT - 1) * 128    # first padded partition in the last tile (32)

_BF16 = ml_dtypes.bfloat16
_BUILD_CACHE = {}


def _build_nc():
    import concourse.bass as bass  # noqa: F401
    import concourse.mybir as mybir
    import concourse.tile as tile
    from concourse import bacc

    f32 = mybir.dt.float32
    bf16 = mybir.dt.bfloat16
    AF = mybir.ActivationFunctionType
    ALU = mybir.AluOpType

    nc = bacc.Bacc("TRN2", target_bir_lowering=False, debug=False)

    q_d = nc.dram_tensor("query", [D, 1], bf16, kind="ExternalInput")
    k_d = nc.dram_tensor("keys", [D, NP], bf16, kind="ExternalInput")
    v_d = nc.dram_tensor("values", [F, NP], bf16, kind="ExternalInput")
    o_d = nc.dram_tensor("out", [F, KK], f32, kind="ExternalOutput")

    with tile.TileContext(nc) as tc:
        with (
            tc.tile_pool(name="const", bufs=1) as constp,
            tc.tile_pool(name="work", bufs=1) as workp,
            tc.tile_pool(name="kring", bufs=3) as kring,
            tc.tile_pool(name="vring", bufs=VRING_BUFS) as vring,
            tc.tile_pool(name="ps_sims", bufs=2, space="PSUM") as ps_sims_p,
            tc.tile_pool(name="ps_small", bufs=2, space="PSUM") as ps_small_p,
            tc.tile_pool(name="ps_out", bufs=1, space="PSUM") as ps_out_p,
        ):
            q_sb = constp.tile([128, 1], bf16)
            nc.sync.dma_start(q_sb[:, :], q_d[:, :])
            ones = constp.tile([128, 128], f32)
            nc.vector.memset(ones[:, :], 1.0)

            sims = workp.tile([128, NT], f32, tag="scrA")
            e0_scr = workp.tile([128, NT], f32, tag="scrB")
            E_a = workp.tile([128, NT], f32)
            E_b = workp.tile([128, NT], f32)
            # t/m scratch reuse the sims/e0 slots (dead after phase-2 setup)
            t_scr = workp.tile([128, NT], f32, tag="scrA")
            m_scr = workp.tile([128, NT], f32, tag="scrB")
            Wp = workp.tile([128, NT, KK + 1], bf16)
            rs = workp.tile([128, 1], f32)
            recip = workp.tile([128, 1], f32)
            sfix = workp.tile([128, 1], f32)
            a_sc = workp.tile([128, 1], f32)
            b_sc = workp.tile([128, 1], f32)
            rvec = workp.tile([128, KK], f32)
            rb_sb = workp.tile([128, KK], f32)
            out17 = workp.tile([128, KK + 1], f32)
            out_sb = workp.tile([128, KK], f32)

            # ----- Value DMA-transposes run on the scalar engine's HWDGE
            # queue, emitted after the phase-2 activations: they fill the
            # vring during phase 2 / phase 3 without stealing HBM bandwidth
            # from the keys stream during phase 1.
            vchunks = [(s, min(VCH, NT - s)) for s in range(0, NT, VCH)]
            vt_tiles = {}
            key_dma_gate = []  # last keys dma inst; set in phase 1

            def emit_vt(ci):
                from concourse.bass import _add_dep_helper

                s, nt_chunk = vchunks[ci]
                vt = vring.tile([128, VCH, 128], bf16, tag="vt")
                vt_tiles[ci] = vt
                ti = nc.scalar.dma_start_transpose(
                    vt[:, 0:nt_chunk, :],
                    v_d[:, s * 128 : (s + nt_chunk) * 128],
                )
                if key_dma_gate:
                    # keep V traffic off the HBM while the keys stream runs
                    _add_dep_helper(
                        ti.ins, key_dma_gate[-1].ins, sync=True,
                        reason="V transpose waits for keys stream",
                    )

            # ---------------- Phase 1: sims ----------------
            ps = None
            for s in range(0, NP, KCH):
                w = min(KCH, NP - s)
                kt = kring.tile([128, KCH], bf16, tag="kt")
                kd = nc.sync.dma_start(kt[:, 0:w], k_d[:, s : s + w])
                if s + w >= NP:
                    key_dma_gate.append(kd)
                for j in range(w // 128):
                    t = s // 128 + j
                    c = t % 512
                    if c == 0:
                        ps = ps_sims_p.tile([128, 512], f32, tag="pss")
                    nc.tensor.matmul(
                        ps[:, c : c + 1],
                        kt[:, j * 128 : (j + 1) * 128],
                        q_sb[:, 0:1],
                        start=True,
                        stop=True,
                    )
                    if c == 511 or t == NT - 1:
                        base = (t // 512) * 512
                        nc.vector.tensor_copy(
                            sims[:, base : t + 1], ps[:, 0 : c + 1]
                        )
            # mark padded positions so exp() kills them (<=32 partitions per
            # memset when base partition is nonzero)
            for p0 in range(PAD_P0, 128, 32):
                nc.vector.memset(sims[p0 : p0 + 32, NT - 1 : NT], 1.0e5)

            # ---------------- Phase 2: iterated softmax ----------------
            # All heavy per-iteration work is on the DVE via the polynomial
            # identity  E_{k+1} = E_k*(1-w)^10 ~= E_k*(1 - 10w + 45w^2),
            # w = E_k/S_k <= ~1e-5 (truncation error ~1e-13, far below f32).
            # ACT only computes e0 and E_1; GpSimd stores W'_k = E_k - 1.
            # e0 = exp(-sims/sqrt(D)); rs = rowsum(e0)
            nc.scalar.activation(
                e0_scr[:, :], sims[:, :], AF.Exp,
                bias=0.0, scale=SIMS_SCALE, accum_out=rs[:, 0:1],
            )
            psS = ps_small_p.tile([128, 1], f32, tag="psS")
            nc.tensor.matmul(psS[:, 0:1], ones[:, :], rs[:, 0:1], start=True, stop=True)
            nc.vector.reciprocal(recip[:, 0:1], psS[:, 0:1])
            # E_1 = exp(10 * e0 / S0): scale AP = 10*r0
            nc.vector.tensor_scalar_mul(sfix[:, 0:1], recip[:, 0:1], 10.0)
            nc.scalar.activation(
                E_a[:, :], e0_scr[:, :], AF.Exp,
                bias=0.0, scale=sfix[:, 0:1], accum_out=rs[:, 0:1],
            )
            # padded positions: E=0 forever (w=0 fixpoint). exp(0)=1 was
            # summed into rs for 96 pad elements -> S_1 fix below.
            for p0 in range(PAD_P0, 128, 32):
                nc.vector.memset(E_a[p0 : p0 + 32, NT - 1 : NT], 0.0)
            # the "sum of V" column
            nc.vector.memset(Wp[:, :, KK], 1.0)

            # V transposes start here: scalar engine is done with compute,
            # phase 2 below is DVE-only.
            for ci in range(len(vchunks)):
                emit_vt(ci)

            cur, nxt = E_a, E_b
            n_pad = 128 - PAD_P0
            for k in range(KK):
                psS = ps_small_p.tile([128, 1], f32, tag="psS")
                nc.tensor.matmul(
                    psS[:, 0:1], ones[:, :], rs[:, 0:1], start=True, stop=True
                )
                # W'_k = E_k - 1 (bf16); DVE does this while the PE reduces,
                # hiding the cross-partition round trip.
                nc.vector.tensor_scalar_add(Wp[:, :, k], cur[:, :], -1.0)
                if k == 0:
                    nc.vector.tensor_scalar_add(sfix[:, 0:1], psS[:, 0:1], -float(n_pad))
                    nc.vector.reciprocal(recip[:, 0:1], sfix[:, 0:1])
                else:
                    nc.vector.reciprocal(recip[:, 0:1], psS[:, 0:1])
                nc.vector.tensor_copy(rvec[0:1, k : k + 1], recip[0:1, 0:1])
                if k < KK - 1:
                    # u = 1 + a*E + b*E^2,  a = -10/S, b = 45/S^2 = 0.45*a*a
                    nc.vector.tensor_scalar_mul(a_sc[:, 0:1], recip[:, 0:1], -10.0)
                    nc.vector.tensor_mul(b_sc[:, 0:1], a_sc[:, 0:1], a_sc[:, 0:1])
                    nc.vector.tensor_scalar_mul(b_sc[:, 0:1], b_sc[:, 0:1], 0.45)
                    nc.vector.tensor_scalar(
                        t_scr[:, :], cur[:, :], b_sc[:, 0:1], a_sc[:, 0:1],
                        op0=ALU.mult, op1=ALU.add,
                    )
                    nc.vector.tensor_mul(m_scr[:, :], t_scr[:, :], cur[:, :])
                    # E_{k+1} = (m+1)*E ; rowsums -> rs
                    nc.vector.scalar_tensor_tensor(
                        nxt[:, :], m_scr[:, :], 1.0, cur[:, :],
                        op0=ALU.add, op1=ALU.mult, accum_out=rs[:, 0:1],
                    )
                    cur, nxt = nxt, cur

            # broadcast 1/S_k across partitions: [128, KK]
            psB = ps_small_p.tile([128, KK], f32, tag="psB")
            nc.tensor.matmul(
                psB[:, :], ones[0:1, :], rvec[0:1, :], start=True, stop=True
            )
            nc.vector.tensor_copy(rb_sb[:, :], psB[:, :])

            # ---------------- Phase 3: weighted sum of values ----------------
            ps_out = ps_out_p.tile([128, KK + 1], f32)
            for ci, (s, nt_chunk) in enumerate(vchunks):
                vt = vt_tiles[ci]
                for j in range(nt_chunk):
                    t = s + j
                    nc.tensor.matmul(
                        ps_out[:, :],
                        vt[:, j, :],
                        Wp[:, t, :],
                        start=(t == 0),
                        stop=(t == NT - 1),
                    )

            # ---------------- Final combine ----------------
            nc.vector.tensor_copy(out17[:, :], ps_out[:, :])
            nc.vector.scalar_tensor_tensor(
                out_sb[:, :],
                out17[:, 0:KK],
                out17[:, KK : KK + 1],
                rb_sb[:, :],
                op0=ALU.add,
                op1=ALU.mult,
            )
            nc.sync.dma_start(o_d[:, :], out_sb[:, :])

    nc.compile()
    return nc


def get_nc():
    if "nc" not in _BUILD_CACHE:
        _BUILD_CACHE["nc"] = _build_nc()
    return _BUILD_CACHE["nc"]


def make_in_maps(query, keys, values):
    in_maps = []
    for b in range(query.shape[0]):
        q = np.ascontiguousarray(query[b].astype(_BF16).reshape(D, 1))
        k = np.zeros((D, NP), _BF16)
        k[:, :N] = keys[b].astype(_BF16)
        v = np.zeros((F, NP), _BF16)
        v[:, :N] = values[b].astype(_BF16)
        in_maps.append({"query": q, "keys": k, "values": v})
    return in_maps


def run(query, keys, values, trace=False):
    nc = get_nc()
    from concourse.bass_utils import run_bass_kernel_spmd

    in_maps = make_in_maps(query, keys, values)
    res = run_bass_kernel_spmd(
        nc, in_maps, core_ids=list(range(N_CORES)), trace=trace
    )
    out = np.stack(
        [np.asarray(r["out"], dtype=np.float32).T for r in res.results], axis=0
    )
    return out, res


def kernel(query, keys, values):
    out, _ = run(query, keys, values, trace=False)
    return out


# revision 19
# speedup vs baseline: 2.2210x; 1.2188x over previous
"""Trainium2 Bass kernel for NeuralKNN (soft k-nearest-neighbors).

Reference computation (per batch element b):
    sims  = -(q . K) / sqrt(D)                      [N]
    a0    = softmax(sims)                           [N]
    repeat 16x:  w_k = softmax(a / 0.1); a += log1p(-w_k)
    out[k, f] = sum_n w_k[n] * V[f, n]              [16, F]

Strategy: pure data-parallel over B=8 -> one batch element per NeuronCore,
no collectives. Per core:
  phase 1: stream K (bf16) through the PE as stationary weights against the
           query vector -> sims laid out [128, 782] (n = t*128 + p).
  phase 2: 16 softmax iterations on [128, 782] in SBUF. Row sums come free
           via activation accum_out; cross-partition sum + broadcast via a
           ones[128,128] matmul. Stores E'_k = exp(10*a)-1 in bf16 (keeps
           precision since E ~= 1) plus a ones column.
  phase 3: V arrives block-transposed via DMA xbar transpose (bf16) as
           [n,f] tiles; one accumulating matmul per 128-n tile:
           psum[f, 0:17] += Vt.T @ [E'_0..E'_15 | 1].
  final:   out[f,k] = (psum[:,k] + psum[:,16]) * (1/S_k); host transposes.

Inputs are cast to bf16 on the host (error through the double-softmax is
~1e-5 relative; value quantization ~0.2% relative, well within tolerance)
and zero-padded from N=100000 to 100096 = 782*128.
"""

import sys

sys.path.insert(0, "/opt/trn_rl_repo")

import numpy as np
import ml_dtypes

B, D, N, F = 8, 128, 100000, 128
KK = 16
TEMP = 0.1
NT = (N + 127) // 128          # 782 n-tiles
NP = NT * 128                  # 100096 padded N
SIMS_SCALE = float(-1.0 / np.sqrt(D))
N_CORES = 8

KCH = 2048                     # keys DMA chunk (columns)
VCH = 48                       # value-transpose DMA chunk (128-col tiles)
VRING_BUFS = 10
PAD_P0 = N - (NT - 1) * 128    # first padded partition in the last tile (32)

_BF16 = ml_dtypes.bfloat16
_BUILD_CACHE = {}


def _build_nc():
    import concourse.bass as bass  # noqa: F401
    import concourse.mybir as mybir
    import concourse.tile as tile
    from concourse import bacc

    f32 = mybir.dt.float32
    bf16 = mybir.dt.bfloat16
    AF = mybir.ActivationFunctionType
    ALU = mybir.AluOpType

    nc = bacc.Bacc("TRN2", target_bir_lowering=False, debug=False)

    q_d = nc.dram_tensor("query", [D, 1], bf16, kind="ExternalInput")
    k_d = nc.dram_tensor("keys", [D, NP], bf16, kind="ExternalInput")
    v_d = nc.dram_tensor("values", [F, NP], bf16, kind="ExternalInput")
    o_d = nc.dram_tensor("out", [F, KK], f32, kind="ExternalOutput")

    with tile.TileContext(nc) as tc:
        with (
            tc.tile_pool(name="const", bufs=1) as constp,
            tc.tile_pool(name="work", bufs=1) as workp,
            tc.tile_pool(name="kring", bufs=6) as kring,
            tc.tile_pool(name="vring", bufs=VRING_BUFS) as vring,
            tc.tile_pool(name="ps_sims", bufs=2, space="PSUM") as ps_sims_p,
            tc.tile_pool(name="ps_small", bufs=2, space="PSUM") as ps_small_p,
            tc.tile_pool(name="ps_out", bufs=1, space="PSUM") as ps_out_p,
        ):
            q_sb = constp.tile([128, 1], bf16)
            nc.sync.dma_start(q_sb[:, :], q_d[:, :])
            ones = constp.tile([128, 128], f32)
            nc.vector.memset(ones[:, :], 1.0)

            sims = workp.tile([128, NT], f32, tag="scrA")
            e0_scr = workp.tile([128, NT], f32, tag="scrB")
            E_a = workp.tile([128, NT], f32)
            E_b = workp.tile([128, NT], f32)
            # t/m scratch reuse the sims/e0 slots (dead after phase-2 setup)
            t_scr = workp.tile([128, NT], f32, tag="scrA")
            m_scr = workp.tile([128, NT], f32, tag="scrB")
            # k-major: W'_k rows are contiguous for fast DVE stores; the
            # phase-3 matmul reads the strided [128, KK+1] column per tile.
            Wp = workp.tile([128, KK + 1, NT], bf16)
            rs = workp.tile([128, 1], f32)
            recip = workp.tile([128, 1], f32)
            sfix = workp.tile([128, 1], f32)
            a_sc = workp.tile([128, 1], f32)
            b_sc = workp.tile([128, 1], f32)
            rvec = workp.tile([128, KK], f32)
            rb_sb = workp.tile([128, KK], f32)
            out17 = workp.tile([128, KK + 1], f32)
            out_sb = workp.tile([128, KK], f32)

            # ----- Value DMA-transposes run on the scalar engine's HWDGE
            # queue, emitted after the phase-2 activations: they fill the
            # vring during phase 2 / phase 3 without stealing HBM bandwidth
            # from the keys stream during phase 1.
            vchunks = [(s, min(VCH, NT - s)) for s in range(0, NT, VCH)]
            vt_tiles = {}
            key_dma_gate = []  # last keys dma inst; set in phase 1

            def emit_vt(ci):
                from concourse.bass import _add_dep_helper

                s, nt_chunk = vchunks[ci]
                vt = vring.tile([128, VCH, 128], bf16, tag="vt")
                vt_tiles[ci] = vt
                # The transpose ucode serializes on its issuing sequencer
                # (~6.4us per chunk) -> alternate the two HWDGE engines.
                eng = nc.scalar
                ti = eng.dma_start_transpose(
                    vt[:, 0:nt_chunk, :],
                    v_d[:, s * 128 : (s + nt_chunk) * 128],
                )
                if key_dma_gate:
                    # keep V traffic off the HBM while the keys stream runs
                    _add_dep_helper(
                        ti.ins, key_dma_gate[-1].ins, sync=True,
                        reason="V transpose waits for keys stream",
                    )

            # ---------------- Phase 1: sims ----------------
            ps = None
            for s in range(0, NP, KCH):
                w = min(KCH, NP - s)
                kt = kring.tile([128, KCH], bf16, tag="kt")
                kd = nc.sync.dma_start(kt[:, 0:w], k_d[:, s : s + w])
                if s + w >= NP:
                    key_dma_gate.append(kd)
                for j in range(w // 128):
                    t = s // 128 + j
                    c = t % 512
                    if c == 0:
                        ps = ps_sims_p.tile([128, 512], f32, tag="pss")
                    nc.tensor.matmul(
                        ps[:, c : c + 1],
                        kt[:, j * 128 : (j + 1) * 128],
                        q_sb[:, 0:1],
                        start=True,
                        stop=True,
                    )
                    if c == 511 or t == NT - 1:
                        base = (t // 512) * 512
                        nc.vector.tensor_copy(
                            sims[:, base : t + 1], ps[:, 0 : c + 1]
                        )
            # mark padded positions so exp() kills them (<=32 partitions per
            # memset when base partition is nonzero)
            for p0 in range(PAD_P0, 128, 32):
                nc.vector.memset(sims[p0 : p0 + 32, NT - 1 : NT], 1.0e5)

            # ---------------- Phase 2: iterated softmax ----------------
            # All heavy per-iteration work is on the DVE via the polynomial
            # identity  E_{k+1} = E_k*(1-w)^10 ~= E_k*(1 - 10w + 45w^2),
            # w = E_k/S_k <= ~1e-5 (truncation error ~1e-13, far below f32).
            # ACT only computes e0 and E_1; GpSimd stores W'_k = E_k - 1.
            # e0 = exp(-sims/sqrt(D)); rs = rowsum(e0)
            nc.scalar.activation(
                e0_scr[:, :], sims[:, :], AF.Exp,
                bias=0.0, scale=SIMS_SCALE, accum_out=rs[:, 0:1],
            )
            psS = ps_small_p.tile([128, 1], f32, tag="psS")
            nc.tensor.matmul(psS[:, 0:1], ones[:, :], rs[:, 0:1], start=True, stop=True)
            nc.vector.reciprocal(recip[:, 0:1], psS[:, 0:1])
            # E_1 = exp(10 * e0 / S0): scale AP = 10*r0
            nc.vector.tensor_scalar_mul(sfix[:, 0:1], recip[:, 0:1], 10.0)
            nc.scalar.activation(
                E_a[:, :], e0_scr[:, :], AF.Exp,
                bias=0.0, scale=sfix[:, 0:1], accum_out=rs[:, 0:1],
            )
            # padded positions: E=0 forever (w=0 fixpoint). exp(0)=1 was
            # summed into rs for 96 pad elements -> S_1 fix below.
            for p0 in range(PAD_P0, 128, 32):
                nc.vector.memset(E_a[p0 : p0 + 32, NT - 1 : NT], 0.0)
            # the "sum of V" column
            nc.vector.memset(Wp[:, KK, :], 1.0)

            # V transposes start here: scalar engine is done with compute,
            # phase 2 below is DVE-only.
            for ci in range(len(vchunks)):
                emit_vt(ci)

            cur, nxt = E_a, E_b
            n_pad = 128 - PAD_P0
            for k in range(KK):
                psS = ps_small_p.tile([128, 1], f32, tag="psS")
                nc.tensor.matmul(
                    psS[:, 0:1], ones[:, :], rs[:, 0:1], start=True, stop=True
                )
                # W'_k = E_k - 1 (bf16); DVE does this while the PE reduces,
                # hiding the cross-partition round trip.
                nc.vector.tensor_scalar_add(Wp[:, k, :], cur[:, :], -1.0)
                if k == 0:
                    nc.vector.tensor_scalar_add(sfix[:, 0:1], psS[:, 0:1], -float(n_pad))
                    nc.vector.reciprocal(recip[:, 0:1], sfix[:, 0:1])
                else:
                    nc.vector.reciprocal(recip[:, 0:1], psS[:, 0:1])
                nc.vector.tensor_copy(rvec[0:1, k : k + 1], recip[0:1, 0:1])
                if k < KK - 1:
                    # u = 1 + a*E + b*E^2,  a = -10/S, b = 45/S^2 = 0.45*a*a
                    nc.vector.tensor_scalar_mul(a_sc[:, 0:1], recip[:, 0:1], -10.0)
                    nc.vector.tensor_mul(b_sc[:, 0:1], a_sc[:, 0:1], a_sc[:, 0:1])
                    nc.vector.tensor_scalar_mul(b_sc[:, 0:1], b_sc[:, 0:1], 0.45)
                    nc.vector.tensor_scalar(
                        t_scr[:, :], cur[:, :], b_sc[:, 0:1], a_sc[:, 0:1],
                        op0=ALU.mult, op1=ALU.add,
                    )
                    nc.vector.tensor_mul(m_scr[:, :], t_scr[:, :], cur[:, :])
                    # E_{k+1} = (m+1)*E ; rowsums -> rs
                    nc.vector.scalar_tensor_tensor(
                        nxt[:, :], m_scr[:, :], 1.0, cur[:, :],
                        op0=ALU.add, op1=ALU.mult, accum_out=rs[:, 0:1],
                    )
                    cur, nxt = nxt, cur

            # broadcast 1/S_k across partitions: [128, KK]
            psB = ps_small_p.tile([128, KK], f32, tag="psB")
            nc.tensor.matmul(
                psB[:, :], ones[0:1, :], rvec[0:1, :], start=True, stop=True
            )
            nc.vector.tensor_copy(rb_sb[:, :], psB[:, :])

            # ---------------- Phase 3: weighted sum of values ----------------
            ps_out = ps_out_p.tile([128, KK + 1], f32)
            for ci, (s, nt_chunk) in enumerate(vchunks):
                vt = vt_tiles[ci]
                for j in range(nt_chunk):
                    t = s + j
                    nc.tensor.matmul(
                        ps_out[:, :],
                        vt[:, j, :],
                        Wp[:, :, t],
                        start=(t == 0),
                        stop=(t == NT - 1),
                    )

            # ---------------- Final combine ----------------
            nc.vector.tensor_copy(out17[:, :], ps_out[:, :])
            nc.vector.scalar_tensor_tensor(
                out_sb[:, :],
                out17[:, 0:KK],
                out17[:, KK : KK + 1],
                rb_sb[:, :],
                op0=ALU.add,
                op1=ALU.mult,
            )
            nc.sync.dma_start(o_d[:, :], out_sb[:, :])

    nc.compile()
    return nc


def get_nc():
    if "nc" not in _BUILD_CACHE:
        _BUILD_CACHE["nc"] = _build_nc()
    return _BUILD_CACHE["nc"]


def make_in_maps(query, keys, values):
    in_maps = []
    for b in range(query.shape[0]):
        q = np.ascontiguousarray(query[b].astype(_BF16).reshape(D, 1))
        k = np.zeros((D, NP), _BF16)
        k[:, :N] = keys[b].astype(_BF16)
        v = np.zeros((F, NP), _BF16)
        v[:, :N] = values[b].astype(_BF16)
        in_maps.append({"query": q, "keys": k, "values": v})
    return in_maps


def run(query, keys, values, trace=False):
    nc = get_nc()
    from concourse.bass_utils import run_bass_kernel_spmd

    in_maps = make_in_maps(query, keys, values)
    res = run_bass_kernel_spmd(
        nc, in_maps, core_ids=list(range(N_CORES)), trace=trace
    )
    out = np.stack(
        [np.asarray(r["out"], dtype=np.float32).T for r in res.results], axis=0
    )
    return out, res


def kernel(query, keys, values):
    out, _ = run(query, keys, values, trace=False)
    return out
